# revision 32
# baseline (speedup 1.0000x reference)
"""Attention2d SPMD kernel for 8 TRN2 NeuronCores.

Problem (hardcoded): x [4, 768, 32, 32], w_qkv [768, 2304], b_qkv [2304],
w_proj [768, 768], b_proj [768]; 32 heads, head_dim 24.

Sharding: 8 cores = 4 batches x 2 query-halves (512 queries each).
Each core computes k/v for all 1024 positions of its batch (2x duplicated
across the pair of cores sharing a batch) and q/attention/proj for its own
512 query positions. Outputs are disjoint slices -> host gather is pure
concatenation (no collectives).

Per-core dataflow:
  k = w_k^T x      [768ch, 1024]  (heads padded 24->32 rows, 4 heads/128)
  q = w_q^T x_q    [768ch, 512]
  vT = x^T w_v     [1024pos, 32 heads x (24 + ones-col)]
  per head: scores_T = k_h^T q_h -> exp (no max-sub; logits in [-7,7])
            o'_h = vT'_h^T exp_sT  (25 rows: 24 channels + denominator)
            o_h  = o'_h * (1/d) via outer-product broadcast matmul + b_v
  out = w_proj^T o + b_proj  [768, 512]
"""

import os
import numpy as np

import concourse.bacc as bacc
import concourse.bass as bass
import concourse.mybir as mybir
import concourse.tile as tile
from concourse import bass_utils

C = 768
HW = 1024
QP = 512          # queries per core
NH = 32           # heads
HD = 24           # head dim
NG = 8            # head groups (4 heads each, 32-padded rows)
CT = C // 128     # 6 contraction tiles
PT = HW // 128    # 8 position tiles
SCALE = HD ** -0.5
F32R = mybir.dt.float32r
BF16 = mybir.dt.bfloat16
F32 = mybir.dt.float32
EXP_BUFS = 8

USE_F32R = os.environ.get("KERNEL_F32", "0") != "1"
KQ_F32R = os.environ.get("KQ_F32R", "0") != "0"  # f32r matmul at row-pos!=0 gives wrong values on HW
KQ_DT = F32R if KQ_F32R else BF16
XW_BF16 = os.environ.get("XW_BF16", "1") != "0"
XW_DT = BF16 if XW_BF16 else F32R


def _r(ap):
    return ap if USE_F32R else ap.bitcast(F32)


def emit_kernel(tc, outs, ins):
    from contextlib import ExitStack
    nc = tc.nc
    ctx = ExitStack()
    Exp = mybir.ActivationFunctionType.Exp

    big = ctx.enter_context(tc.tile_pool(name="big", bufs=1))
    kqp = ctx.enter_context(tc.tile_pool(name="kqp", bufs=2))
    wgp = ctx.enter_context(tc.tile_pool(name="wgp", bufs=3))
    expp = ctx.enter_context(tc.tile_pool(name="expp", bufs=EXP_BUFS))
    smal = ctx.enter_context(tc.tile_pool(name="smal", bufs=2))
    outp = ctx.enter_context(tc.tile_pool(name="outp", bufs=2))
    ps_gen = ctx.enter_context(tc.tile_pool(name="ps_gen", bufs=3, space="PSUM"))
    ps_s = ctx.enter_context(tc.tile_pool(name="ps_s", bufs=2, space="PSUM"))
    ps_o = ctx.enter_context(tc.tile_pool(name="ps_o", bufs=1, space="PSUM"))

    # ---------------- persistent SBUF tensors ----------------
    x_sb = big.tile([128, CT, HW], XW_DT)
    wv_sb = big.tile([128, CT, C], XW_DT)
    wp_sb = big.tile([128, NG, C], F32R)           # 3 MB
    vt_sb = big.tile([128, PT, NH, 32], BF16)      # 2 MB
    opad_sb = big.tile([128, NG, QP], F32R)        # 2 MB
    bk_sb = big.tile([128, NG], F32)
    bq_sb = big.tile([128, NG], F32)
    bv_sb = big.tile([128, NG], F32)
    bp_sb = big.tile([128, CT], F32)

    xv = ins["x"].rearrange("(t p) n -> p t n", p=128)
    wvv = ins["wv"].rearrange("(t p) m -> p t m", p=128)
    for ct in range(CT):
        nc.sync.dma_start(out=x_sb[:, ct, :], in_=xv[:, ct, :])
        nc.sync.dma_start(out=wv_sb[:, ct, :], in_=wvv[:, ct, :])
    nc.sync.dma_start(out=bk_sb, in_=ins["bk"])
    nc.sync.dma_start(out=bq_sb, in_=ins["bq"])
    nc.sync.dma_start(out=bv_sb, in_=ins["bv"])
    nc.sync.dma_start(out=bp_sb, in_=ins["bp"])
    nc.sync.dma_start(out=vt_sb[:, :, :, HD:32], in_=ins["vinit"])

    def emit_vt_half(t):
        # vT for heads 16t..16t+16 (dense, N=384) over all 8 pos tiles
        for pt in range(PT):
            vps = ps_gen.tile([128, 384], F32, tag="gen")
            for ct in range(CT):
                nc.tensor.matmul(
                    vps[:, :],
                    lhsT=_r(x_sb[:, ct, pt * 128:(pt + 1) * 128]),
                    rhs=_r(wv_sb[:, ct, 384 * t:384 * (t + 1)]),
                    start=(ct == 0), stop=(ct == CT - 1),
                )
            nc.vector.tensor_copy(
                out=vt_sb[:, pt, 16 * t:16 * (t + 1), 0:HD],
                in_=vps.rearrange("p (h d) -> p h d", d=HD),
            )

    emit_vt_half(0)
    pps_early = []

    # ---------------- per head-group: kq proj + attention ----------
    for g in range(NG):
        wkq = wgp.tile([128, CT, 256], XW_DT, tag="wkq")
        nc.sync.dma_start(out=wkq, in_=ins["wkq"][g])
        wkg = wkq[:, :, 0:128]
        wqg = wkq[:, :, 128:256]

        kg_sb = kqp.tile([128, HW], KQ_DT, tag="kg")
        qg_sb = kqp.tile([128, QP], KQ_DT, tag="qg")
        for half in range(2):
            kps = ps_gen.tile([128, 512], F32, tag="gen")
            for ct in range(CT):
                nc.tensor.matmul(
                    kps[:, :],
                    lhsT=_r(wkg[:, ct, :]),
                    rhs=_r(x_sb[:, ct, half * 512:(half + 1) * 512]),
                    start=(ct == 0), stop=(ct == CT - 1),
                )
            nc.vector.tensor_scalar_add(
                kg_sb[:, half * 512:(half + 1) * 512], kps, bk_sb[:, g:g + 1])
        qps = ps_gen.tile([128, 512], F32, tag="gen")
        for ct in range(CT):
            nc.tensor.matmul(
                qps[:, :],
                lhsT=_r(wqg[:, ct, :]),
                rhs=_r(x_sb[:, ct, 0:QP]),
                start=(ct == 0), stop=(ct == CT - 1),
            )
        nc.vector.tensor_scalar_add(qg_sb[:, :], qps, bq_sb[:, g:g + 1])
        if KQ_F32R:
            k3_sb = kqp.tile([32, HW], KQ_DT, tag="k3")
            q3_sb = kqp.tile([32, QP], KQ_DT, tag="q3")
            nc.sync.dma_start(out=k3_sb[0:HD, :], in_=kg_sb[96:96 + HD, :])
            nc.sync.dma_start(out=q3_sb[0:HD, :], in_=qg_sb[96:96 + HD, :])

        if g == 1:
            emit_vt_half(1)
        if g == 2:
            for ct in range(NG):
                nc.sync.dma_start(
                    out=wp_sb[:, ct, :],
                    in_=ins["wp"].rearrange("(t p) m -> p t m", p=128)[:, ct, :])
        if g == NG - 1:
            for ft in range(len(pps_early)):
                pps = pps_early[ft]
                for ct in range(NG - 1):
                    nc.tensor.matmul(
                        pps[:, :],
                        lhsT=_r(wp_sb[:, ct, ft * 128:(ft + 1) * 128]),
                        rhs=_r(opad_sb[:, ct, :]),
                        start=(ct == 0), stop=False,
                    )

        o_ps = ps_o.tile([128, QP], F32, tag="ops")
        o_sb = smal.tile([128, QP], F32, tag="osb")
        for j in range(4):
            h = 4 * g + j
            b0 = 32 * j
            if j == 3 and KQ_F32R:
                s_lhs, s_rhs, s_b0 = k3_sb, q3_sb, 0
            else:
                s_lhs, s_rhs, s_b0 = kg_sb, qg_sb, b0
            for kp in range(PT // 2):
                sps = ps_s.tile([128, 2, QP], F32, tag="sps")
                for i in range(2):
                    kt = 2 * kp + i
                    nc.tensor.matmul(
                        sps[:, i, :],
                        lhsT=_r(s_lhs[s_b0:s_b0 + HD, kt * 128:(kt + 1) * 128]),
                        rhs=_r(s_rhs[s_b0:s_b0 + HD, :]),
                        start=True, stop=True, tile_position=(s_b0, 0),
                    )
                et = expp.tile([128, 2, QP], BF16, tag="exp")
                nc.scalar.activation(et[:, :, :], sps[:, :, :], Exp, scale=SCALE)
                for i in range(2):
                    kt = 2 * kp + i
                    nc.tensor.matmul(
                        o_ps[b0:b0 + 32, :],
                        lhsT=_r(vt_sb[:, kt, h, :]),
                        rhs=_r(et[:, i, :]),
                        start=(kt == 0), stop=(kt == PT - 1), tile_position=(0, b0),
                    )
            if j == 3:
                nc.vector.tensor_copy(out=o_sb[:, :], in_=o_ps[:, :])

        # denominators: d rows -> DRAM bounce -> stride-0 broadcast back,
        # fp32 reciprocal, exact fp32 division (per 32-row head block)
        rc1 = smal.tile([128, QP], F32, tag="rc1")
        for j in range(4):
            nc.sync.dma_start(out=ins["dscr"][g, j].unsqueeze(0), in_=o_sb[32 * j + HD:32 * j + HD + 1, :])
            nc.sync.dma_start(out=rc1[32 * j:32 * (j + 1), :],
                              in_=ins["dscr"][g, j].unsqueeze(0).to_broadcast((32, QP)))
        rcf = smal.tile([128, QP], F32, tag="rcf")
        nc.vector.reciprocal(rcf[:, :], rc1[:, :])
        for j in range(4):
            b0 = 32 * j
            nc.vector.tensor_mul(
                opad_sb[b0:b0 + 32, g, :], o_sb[b0:b0 + 32, :], rcf[b0:b0 + 32, :])
            nc.gpsimd.tensor_scalar_add(
                opad_sb[b0:b0 + 32, g, :], opad_sb[b0:b0 + 32, g, :],
                bv_sb[b0:b0 + 32, g:g + 1])

    # ---------------- out = w_proj^T o + b_proj ----------------
    # (ft 0..1 were partially accumulated during group 7; finish them first)
    for ft in range(CT):
        if ft < len(pps_early):
            pps = pps_early[ft]
            nc.tensor.matmul(
                pps[:, :],
                lhsT=_r(wp_sb[:, NG - 1, ft * 128:(ft + 1) * 128]),
                rhs=_r(opad_sb[:, NG - 1, :]),
                start=False, stop=True,
            )
        else:
            pps = ps_gen.tile([128, QP], F32, tag="gen")
            for ct in range(NG):
                nc.tensor.matmul(
                    pps[:, :],
                    lhsT=_r(wp_sb[:, ct, ft * 128:(ft + 1) * 128]),
                    rhs=_r(opad_sb[:, ct, :]),
                    start=(ct == 0), stop=(ct == NG - 1),
                )
        out_t = outp.tile([128, QP], F32, tag="out")
        nc.vector.tensor_scalar_add(out_t[:, :], pps, bp_sb[:, ft:ft + 1])
        nc.sync.dma_start(
            out=outs["out"].rearrange("(t p) q -> t p q", p=128)[ft], in_=out_t)

    ctx.close()


# ------------------------- host side -------------------------

def build_inmaps(x, w_qkv, b_qkv, w_proj, b_proj):
    x = np.ascontiguousarray(x, dtype=np.float32)
    w_qkv = np.asarray(w_qkv, dtype=np.float32)
    b_qkv = np.asarray(b_qkv, dtype=np.float32)
    w_proj = np.asarray(w_proj, dtype=np.float32)
    b_proj = np.asarray(b_proj, dtype=np.float32)

    w_q, w_k, w_v = w_qkv[:, :C], w_qkv[:, C:2 * C], w_qkv[:, 2 * C:]
    b_q, b_k, b_v = b_qkv[:C], b_qkv[C:2 * C], b_qkv[2 * C:]

    def pad_w(w):  # [768, 768] -> [768, 1024] with 24->32 head col padding
        out = np.zeros((C, NH, 32), dtype=np.float32)
        out[:, :, :HD] = w.reshape(C, NH, HD)
        return out.reshape(C, NH * 32)

    def pad_b(b):  # [768] -> [128, 8]
        out = np.zeros((4, 32, NG), dtype=np.float32)
        out[:, :HD, :] = b.reshape(NG, 4, HD).transpose(1, 2, 0)
        return out.reshape(128, NG)

    import ml_dtypes
    xw_dt = ml_dtypes.bfloat16 if XW_BF16 else np.float32
    wk_g = pad_w(w_k).reshape(C, NG, 128).transpose(1, 0, 2)   # [NG, C, 128]
    wq_g = pad_w(w_q).reshape(C, NG, 128).transpose(1, 0, 2)
    wkq = np.concatenate([wk_g, wq_g], axis=2)                 # [NG, C, 256]
    # preswizzle to [NG, 128, CT, 256] so each partition's DMA read is contiguous
    wkq = np.ascontiguousarray(
        wkq.reshape(NG, CT, 128, 256).transpose(0, 2, 1, 3)).astype(xw_dt)
    wp_pad = np.zeros((NH, 32, C), dtype=np.float32)
    wp_pad[:, :HD, :] = w_proj.reshape(NH, HD, C)
    wp_pad = wp_pad.reshape(NH * 32, C)
    bk = pad_b(b_k)
    bq = pad_b(b_q)
    bv = pad_b(b_v)
    bp = np.ascontiguousarray(b_proj.reshape(CT, 128).T)
    vinit = np.zeros((128, PT, NH, 8), dtype=ml_dtypes.bfloat16)
    vinit[:, :, :, 0] = 1.0

    in_maps = []
    for core in range(8):
        b, half = core // 2, core % 2
        xb = x[b].reshape(C, HW)
        # rotate so this core's queries are always columns 0:QP (keys are
        # permutation-invariant under softmax)
        xb = np.ascontiguousarray(np.roll(xb, -half * QP, axis=1)).astype(xw_dt)
        in_maps.append({
            "x": xb,
            "wkq": wkq,
            "wv": np.ascontiguousarray(w_v).astype(xw_dt),
            "wp": wp_pad,
            "bk": bk, "bq": bq, "bv": bv, "bp": bp,
            "vinit": vinit,
        })
    return in_maps


_PROGRAM = None


def build_program():
    global _PROGRAM
    if _PROGRAM is not None:
        return _PROGRAM
    nc = bacc.Bacc("TRN2", target_bir_lowering=False, debug=False)
    ins = {
        "x": nc.dram_tensor("x", [C, HW], XW_DT, kind="ExternalInput").ap(),
        "wkq": nc.dram_tensor("wkq", [NG, 128, CT, 256], XW_DT, kind="ExternalInput").ap(),
        "wv": nc.dram_tensor("wv", [C, C], XW_DT, kind="ExternalInput").ap(),
        "wp": nc.dram_tensor("wp", [1024, C], F32R, kind="ExternalInput").ap(),
        "bk": nc.dram_tensor("bk", [128, NG], F32, kind="ExternalInput").ap(),
        "bq": nc.dram_tensor("bq", [128, NG], F32, kind="ExternalInput").ap(),
        "bv": nc.dram_tensor("bv", [128, NG], F32, kind="ExternalInput").ap(),
        "bp": nc.dram_tensor("bp", [128, CT], F32, kind="ExternalInput").ap(),
        "vinit": nc.dram_tensor("vinit", [128, PT, NH, 8], BF16, kind="ExternalInput").ap(),
    }
    ins["dscr"] = nc.dram_tensor("dscr", [NG, 4, QP], F32).ap()
    outs = {"out": nc.dram_tensor("out", [C, QP], F32, kind="ExternalOutput").ap()}
    with tile.TileContext(nc) as tc:
        emit_kernel(tc, outs, ins)
    nc.compile()
    _PROGRAM = nc
    return nc


def run(inputs, trace=False):
    nc = build_program()
    in_maps = build_inmaps(**inputs)
    res = bass_utils.run_bass_kernel_spmd(
        nc, in_maps, core_ids=list(range(8)), trace=trace)
    out_full = np.empty((4, C, HW), dtype=np.float32)
    for core in range(8):
        b, half = core // 2, core % 2
        out_full[b][:, half * QP:(half + 1) * QP] = res.results[core]["out"]
    return out_full.reshape(4, C, 32, 32), res


def kernel(**inputs):
    out, _ = run(inputs, trace=False)
    return out
